# revision 1
# baseline (speedup 1.0000x reference)
"""DeepSeekV3 latent attention (MLA) Trainium2 Bass kernel.

Sharding: 8 cores = 2 batches x 4 head-groups (4 heads each).
Each core computes, for its (batch b, head-group hg):
  - c_kv = RMSNorm(x_b @ W_DKV.T) * w        (replicated across the 4 hg cores)
  - k_rope / q projections for its 4 heads (weights sliced on the head axis)
  - causal latent attention (no-max-sub softmax, exp/sum form)
  - out_partial = ctx_hg @ out_w[:, hg_cols].T   (row-parallel partial)
Host sums the 4 partials per batch and adds the bias.

Final layout vs the original baseline (measured 820us -> ~738us):
  - Phase B runs query-supertile (j) OUTER, head INNER, with the absorbed-q
    (qa) matmuls and the output projection fused into the same dense stream,
    and the whole thing software-pipelined: each context's PSUM drains
    (denominator chain, ctx-latent copies, UV) are emitted behind the NEXT
    context's qa + first QK quad so the PE FIFO never heads into them.
    This keeps PE density high everywhere and the HAM clock gate at 8/8.
  - Context emission order is j1, j0, j2, j3: j0's shallow contexts sit at
    the tail of the Phase-A weave where deep j2 quads cover their drains.
  - The softmax denominator row is broadcast across partitions with a rank-1
    fp32r matmul instead of a DRAM round trip; reciprocal runs out of SBUF
    after a fast scalar drain of the PSUM bank.
  - ckvT is produced by ONE batched xbar DMA-transpose per 128-token tile
    into a contiguous [128, NT, LC, 128] destination.
  - Weight DMAs are batched 4-dc-per-issue and the attention-phase weights
    stay EARLY on the gpsimd queue (anything queued between them and the
    projection weights delays attention start by ~100us).
  - Output partials are bf16 (halves the output DMA).
  - Copy/elementwise work is split between ScalarE and VectorE so neither
    FIFO stalls the PE; nothing latency-critical runs on GpSimd.
Device layout: feature-on-partition, token-on-free throughout, so scores
come out as S^T [tk, tq] and probs feed the PV matmul with no transposes.
"""

import numpy as np
import ml_dtypes

import concourse.bass as bass
import concourse.tile as tile
from concourse import bacc
from concourse import mybir
from concourse.bass import ts
from concourse.bass_utils import run_bass_kernel_spmd

BF16 = mybir.dt.bfloat16
F32 = mybir.dt.float32
F32R = mybir.dt.float32r
NPBF16 = ml_dtypes.bfloat16

H, HD, RD, LAT = 16, 128, 64, 512
D_IN = 2048
D_OUT = H * HD
HPC = 4  # heads per core
LC = LAT // 128
EPS = 1e-6
THETA = 10000.0
SCALE = 1.0 / float(np.sqrt(np.float32(HD + RD)))
AF = mybir.ActivationFunctionType
ALU = mybir.AluOpType


def build_mla_nc(T=2048):
    nc = bacc.Bacc("TRN2", target_bir_lowering=False)
    DC = D_IN // 128      # 16 contraction chunks for the x projections
    NT = T // 128         # 128-token tiles
    NJ = T // 512         # 512-token query supertiles

    # ---------------- I/O (all layouts are host-prepared, partition-major) ---
    xT = nc.dram_tensor("xT", [128, DC, T], BF16, kind="ExternalInput")
    wdkvT = nc.dram_tensor("wdkvT", [128, DC, LAT], BF16, kind="ExternalInput")
    wkrT = nc.dram_tensor("wkrT", [128, DC, HPC * RD], BF16, kind="ExternalInput")
    wqcT = nc.dram_tensor("wqcT", [128, DC, HPC * HD], BF16, kind="ExternalInput")
    wqrT = nc.dram_tensor("wqrT", [128, DC, HPC * RD], BF16, kind="ExternalInput")
    wuk = nc.dram_tensor("wuk", [128, HPC, LAT], BF16, kind="ExternalInput")
    wuvT = nc.dram_tensor("wuvT", [128, HPC, LC, HD], BF16, kind="ExternalInput")
    owT = nc.dram_tensor("owT", [128, HPC, D_OUT], BF16, kind="ExternalInput")
    kvw = nc.dram_tensor("kvw", [128, LAT], F32, kind="ExternalInput")
    cosT = nc.dram_tensor("cosT", [128, T], BF16, kind="ExternalInput")
    sinT = nc.dram_tensor("sinT", [128, T], BF16, kind="ExternalInput")
    perm = nc.dram_tensor("perm", [128, 128], BF16, kind="ExternalInput")
    masks = nc.dram_tensor("masks", [128, 4, 512], BF16, kind="ExternalInput")
    ones1 = nc.dram_tensor("ones1", [128, 1], BF16, kind="ExternalInput")
    onesr = nc.dram_tensor("onesr", [1, 128], F32R, kind="ExternalInput")
    out_p = nc.dram_tensor("out_p", [T, D_OUT], BF16, kind="ExternalOutput")

    with tile.TileContext(nc) as tc:
        with tc.tile_pool(name="persist", bufs=1) as persist:
            # persistent activations
            ckv_nat = persist.tile([128, NT, LAT], BF16)   # [t%128, ttile, lat]
            ckvT = persist.tile([128, NT, LC, 128], BF16)  # [lat%128, ttile, lc, tok]
            kTrot = persist.tile([128, 2, T], BF16)        # [pairrow, h//2, t]
            qTrot = persist.tile([128, 2, T], BF16)
            qcT = persist.tile([128, HPC, T], BF16)        # [hd, h, t]
            ctxT = persist.tile([128, HPC, T], BF16)       # [hd, h, t]

            # ============== Phase A: projections + RMSNorm + RoPE ===========
            with (
                tc.tile_pool(name="ps_a", bufs=8, space="PSUM") as ps_a,
                tc.tile_pool(name="aw", bufs=1) as aw,
                tc.tile_pool(name="xs", bufs=2) as xs,
                tc.tile_pool(name="cs", bufs=2) as cs,
                tc.tile_pool(name="wka", bufs=3) as wka,
            ):
                wdkvT_s = aw.tile([128, DC, LAT], BF16)
                wkrT_s = aw.tile([128, DC, HPC * RD], BF16)
                wqcT_s = aw.tile([128, DC, HPC * HD], BF16)
                wqrT_s = aw.tile([128, DC, HPC * RD], BF16)
                for d4 in range(DC // 4):
                    sl = slice(4 * d4, 4 * d4 + 4)
                    nc.gpsimd.dma_start(wdkvT_s[:, sl, :], wdkvT[:, sl, :])
                for d4 in range(DC // 4):
                    sl = slice(4 * d4, 4 * d4 + 4)
                    nc.gpsimd.dma_start(wkrT_s[:, sl, :], wkrT[:, sl, :])
                    nc.gpsimd.dma_start(wqcT_s[:, sl, :], wqcT[:, sl, :])
                    nc.gpsimd.dma_start(wqrT_s[:, sl, :], wqrT[:, sl, :])
                perm_s = aw.tile([128, 128], BF16)
                nc.gpsimd.dma_start(perm_s, perm[:, :])
                kvw_s = aw.tile([128, LAT], F32)
                nc.gpsimd.dma_start(kvw_s, kvw[:, :])
                eps_s = aw.tile([128, 1], F32)
                nc.vector.memset(eps_s, EPS)

                def rope_pair(raw, dst, rc, jt, cos_s, sin_s, tag):
                    # raw: [128,512] sbuf with 2 heads' raw rope rows.
                    psr = ps_a.tile([128, 512], F32, tag="mm")
                    nc.tensor.matmul(psr, lhsT=perm_s, rhs=raw, start=True, stop=True)
                    # drain the PSUM bank immediately (scalar) so the 8-deep
                    # ps_a rotation never stalls on the deep DVE queue
                    rot = wka.tile([128, 512], BF16, tag=f"{tag}_rot")
                    nc.scalar.copy(rot, psr)
                    tmp = wka.tile([128, 512], BF16, tag=f"{tag}_cos")
                    nc.vector.tensor_mul(tmp, raw, cos_s)
                    tmp2 = wka.tile([128, 512], BF16, tag=f"{tag}_sin")
                    nc.vector.tensor_mul(tmp2, rot, sin_s)
                    nc.vector.tensor_add(dst[:, rc, ts(jt, 512)], tmp, tmp2)

                rope_pending = []

                def flush_rope():
                    while rope_pending:
                        rope_pair(*rope_pending.pop(0))

                for jt in range(NJ):
                    xts = xs.tile([128, DC, 512], BF16)
                    for q4 in range(4):
                        nc.scalar.dma_start(
                            xts[:, 4 * q4 : 4 * q4 + 4, :],
                            xT[:, 4 * q4 : 4 * q4 + 4, ts(jt, 512)],
                        )
                    cos_s = cs.tile([128, 512], BF16, tag="cos")
                    nc.scalar.dma_start(cos_s, cosT[:, ts(jt, 512)])
                    sin_s = cs.tile([128, 512], BF16, tag="sin")
                    nc.scalar.dma_start(sin_s, sinT[:, ts(jt, 512)])

                    # --- c_kv (natural layout) + RMSNorm ---
                    for tt4 in range(4):
                        tt = jt * 4 + tt4
                        ps = ps_a.tile([128, 512], F32, tag="mm")
                        for dc in range(DC):
                            nc.tensor.matmul(
                                ps,
                                lhsT=xts[:, dc, ts(tt4, 128)],
                                rhs=wdkvT_s[:, dc, :],
                                start=(dc == 0),
                                stop=(dc == DC - 1),
                            )
                        sq = wka.tile([128, LAT], BF16, tag="sq")
                        ssum = wka.tile([128, 1], F32, tag="ssum")
                        nc.scalar.activation(sq, ps, AF.Square, accum_out=ssum)
                        rstd = wka.tile([128, 1], F32, tag="rstd")
                        nc.scalar.activation(
                            rstd, ssum, AF.Sqrt, bias=eps_s, scale=1.0 / LAT
                        )
                        nc.vector.reciprocal(rstd, rstd)
                        nc.vector.scalar_tensor_tensor(
                            ckv_nat[:, tt, :], ps, rstd, kvw_s,
                            op0=ALU.mult, op1=ALU.mult,
                        )
                        # transposed copy for the QK side (single xbar
                        # transpose per token tile; contiguous destination)
                        nc.sync.dma_start_transpose(
                            ckvT[:, tt, :, :], ckv_nat[:, tt, :]
                        )

                    # --- rope + q projections, rc0 (heads 0/1) first so the
                    #     woven attention contexts unblock as early as possible
                    def proj_group(w_s, col):
                        ps = ps_a.tile([128, 512], F32, tag="mm")
                        for dc in range(DC):
                            nc.tensor.matmul(
                                ps,
                                lhsT=w_s[:, dc, ts(col, 128)],
                                rhs=xts[:, dc, :],
                                start=(dc == 0),
                                stop=(dc == DC - 1),
                            )
                        return ps

                    for rc in range(2):
                        ps = proj_group(wkrT_s, rc)
                        raw = wka.tile([128, 512], BF16, tag="k_raw")
                        nc.scalar.copy(raw, ps)
                        rope_pending.append((raw, kTrot, rc, jt, cos_s, sin_s, "k"))
                        ps = proj_group(wqrT_s, rc)
                        raw = wka.tile([128, 512], BF16, tag="q_raw")
                        nc.scalar.copy(raw, ps)
                        rope_pending.append((raw, qTrot, rc, jt, cos_s, sin_s, "q"))
                        for fc in (range(2) if rc == 0 else range(2, HPC)):
                            ps = proj_group(wqcT_s, fc)
                            nc.scalar.copy(qcT[:, fc, ts(jt, 512)], ps)
                            flush_rope()
                flush_rope()

            # ============== Phase B: attention, j outer / head inner ========
            with (
                tc.tile_pool(name="ps_s", bufs=3, space="PSUM") as ps_s,
                tc.tile_pool(name="ps_pv", bufs=1, space="PSUM") as ps_pv,
                tc.tile_pool(name="ps_dn", bufs=1, space="PSUM") as ps_dn,
                tc.tile_pool(name="bw", bufs=1) as bw,
                tc.tile_pool(name="qa", bufs=3) as qa_pool,
                tc.tile_pool(name="exps", bufs=12) as exps,
                tc.tile_pool(name="wkb", bufs=3) as wkb,
                tc.tile_pool(name="outs", bufs=2) as outs,
            ):
                wuk_s = bw.tile([128, HPC, LAT], BF16)
                nc.gpsimd.dma_start(wuk_s, wuk[:, :, :])
                wuvT_s = bw.tile([128, HPC, LC, HD], BF16)
                nc.gpsimd.dma_start(wuvT_s, wuvT[:, :, :, :])
                masks_s = bw.tile([128, 4, 512], BF16)
                nc.gpsimd.dma_start(masks_s, masks[:, :, :])
                ones1_s = bw.tile([128, 1], BF16)
                nc.gpsimd.dma_start(ones1_s, ones1[:, :])
                onesr_s = bw.tile([1, 128], F32R)
                nc.gpsimd.dma_start(onesr_s, onesr[:, :])
                owT_s = bw.tile([128, HPC, D_OUT], BF16)
                for hc4 in range(HPC):
                    nc.gpsimd.dma_start(owT_s[:, hc4, :], owT[:, hc4, :])

                def emit_qa(j, h):
                    qa_t = qa_pool.tile([128, LC, 512], BF16, tag="qa")
                    for lc in range(LC):
                        ps = ps_s.tile([128, 512], F32, tag="sc")
                        nc.tensor.matmul(
                            ps,
                            lhsT=wuk_s[:, h, ts(lc, 128)],
                            rhs=qcT[:, h, ts(j, 512)],
                            start=True,
                            stop=True,
                        )
                        nc.scalar.copy(qa_t[:, lc, :], ps)
                    return qa_t

                def emit_qk_quad(j, h, quad, qa_t):
                    nquad = j + 1
                    hb = (h % 2) * 64
                    rc = h // 2
                    exs = []
                    for tq in range(4):
                        tk = 4 * quad + tq
                        ps = ps_s.tile([128, 512], F32, tag="sc")
                        for lc in range(LC):
                            nc.tensor.matmul(
                                ps,
                                lhsT=ckvT[:, tk, lc, :],
                                rhs=qa_t[:, lc, :],
                                start=(lc == 0),
                                stop=False,
                            )
                        nc.tensor.matmul(
                            ps,
                            lhsT=kTrot[hb : hb + 64, rc, ts(tk, 128)],
                            rhs=qTrot[hb : hb + 64, rc, ts(j, 512)],
                            start=False,
                            stop=True,
                        )
                        ex = exps.tile([128, 512], BF16, tag="exp")
                        nc.scalar.activation(ex, ps, AF.Exp, scale=SCALE)
                        if quad == nquad - 1:
                            nc.vector.tensor_mul(ex, ex, masks_s[:, tq, :])
                        exs.append(ex)
                    return exs

                def emit_pv_quad(j, quad, exs, ps_ctx, ps_d):
                    ntk = 4 * (j + 1)
                    for tq in range(4):
                        tk = 4 * quad + tq
                        for lc in range(LC):
                            nc.tensor.matmul(
                                ps_ctx[:, lc, :],
                                lhsT=ckv_nat[:, tk, ts(lc, 128)],
                                rhs=exs[tq],
                                start=(tk == 0),
                                stop=(tk == ntk - 1),
                            )
                        nc.tensor.matmul(
                            ps_d[0:1, :],
                            lhsT=ones1_s,
                            rhs=exs[tq],
                            start=(tk == 0),
                            stop=(tk == ntk - 1),
                        )

                def emit_dn_copy(ps_d):
                    # dependency-free scalar copy of the denominator row; goes
                    # first so it clears the scalar FIFO before the qa copies
                    dn_sb = wkb.tile([1, 512], F32R, tag="dn_sb")
                    nc.scalar.copy(dn_sb, ps_d[0:1, :])
                    return dn_sb

                def emit_dn_bcast(dn_sb):
                    # rank-1 fp32r broadcast; bank drained fast by a scalar
                    # copy.  The reciprocal is emitted later (inside
                    # emit_drain) so the cl copies lead the DVE FIFO.
                    ps_bc = ps_s.tile([128, 512], F32, tag="sc")
                    nc.tensor.matmul(
                        ps_bc, lhsT=onesr_s, rhs=dn_sb, start=True, stop=True
                    )
                    dbr = wkb.tile([128, 512], F32, tag="dbr")
                    nc.scalar.copy(dbr, ps_bc)
                    return dbr

                def emit_drain(j, h, ps_ctx, dbr):
                    cl = wkb.tile([128, LC, 512], BF16, tag="ctxlat")
                    for lc in range(LC):
                        nc.vector.tensor_copy(cl[:, lc, :], ps_ctx[:, lc, :])
                    db = wkb.tile([128, 512], F32, tag="db")
                    nc.vector.reciprocal(db, dbr)
                    ps_uv = ps_s.tile([128, 512], F32, tag="sc")
                    for lc in range(LC):
                        nc.tensor.matmul(
                            ps_uv,
                            lhsT=wuvT_s[:, h, lc, :],
                            rhs=cl[:, lc, :],
                            start=(lc == 0),
                            stop=(lc == LC - 1),
                        )
                    nc.vector.tensor_mul(ctxT[:, h, ts(j, 512)], ps_uv, db)  # db is ps_bc (PSUM)

                def emit_out_proj(j):
                    for tt4 in range(4):
                        tt = 4 * j + tt4
                        ot = outs.tile([128, D_OUT], BF16, tag="ot")
                        for oc in range(D_OUT // 512):
                            ps = ps_s.tile([128, 512], F32, tag="sc")
                            for hc in range(HPC):
                                nc.tensor.matmul(
                                    ps,
                                    lhsT=ctxT[:, hc, ts(tt, 128)],
                                    rhs=owT_s[:, hc, ts(oc, 512)],
                                    start=(hc == 0),
                                    stop=(hc == HPC - 1),
                                )
                            if oc % 2 == 0:
                                nc.scalar.copy(ot[:, ts(oc, 512)], ps)
                            else:
                                nc.vector.tensor_copy(ot[:, ts(oc, 512)], ps)
                            nc.sync.dma_start(
                                out_p[ts(tt, 128), ts(oc, 512)], ot[:, ts(oc, 512)]
                            )

                # software-pipelined emission: each context's drain (and each
                # j's out-proj) is deferred until after the next context's
                # qa + first QK quad, so the PE FIFO never heads into a
                # dependency on the slow PSUM-drain chains.
                # j1 before j0: j0's shallow contexts then sit at the tail
                # of the Phase-A weave where their drains are covered by j2's
                # deep QK quads instead of being exposed at the A/B seam
                j_order = [1, 0] + list(range(2, NJ))
                contexts = [(j, h) for j in j_order for h in range(HPC)]
                pending = None   # (j, h, ps_ctx, db)
                pending_out = None
                for (j, h) in contexts:
                    if pending is not None:
                        dn_sb = emit_dn_copy(pending_psd)
                    qa_t = emit_qa(j, h)
                    if pending is not None:
                        pending = (*pending[:3], emit_dn_bcast(dn_sb))
                    exs0 = emit_qk_quad(j, h, 0, qa_t)
                    if pending_out is not None:
                        emit_out_proj(pending_out)
                        pending_out = None
                    if pending is not None:
                        emit_drain(pending[0], pending[1], pending[2], pending[3])
                        if pending[1] == HPC - 1:
                            pending_out = pending[0]
                        pending = None
                    ps_ctx = ps_pv.tile([128, LC, 512], F32, tag="pv")
                    ps_d = ps_dn.tile([128, 512], F32, tag="dn")
                    emit_pv_quad(j, 0, exs0, ps_ctx, ps_d)
                    for quad in range(1, j + 1):
                        exs = emit_qk_quad(j, h, quad, qa_t)
                        emit_pv_quad(j, quad, exs, ps_ctx, ps_d)
                    pending = (j, h, ps_ctx, None)
                    pending_psd = ps_d
                dn_sb = emit_dn_copy(pending_psd)
                dbr = emit_dn_bcast(dn_sb)
                emit_drain(pending[0], pending[1], pending[2], dbr)
                emit_out_proj(NJ - 1)

    nc.finalize()
    return nc


def _part_major(a2d):
    """[R, C] -> [128, R//128, C] with partition = R % 128."""
    r, c = a2d.shape
    return np.ascontiguousarray(
        a2d.reshape(r // 128, 128, c).transpose(1, 0, 2)
    )


def make_in_maps(x, W_DKV, kv_norm_w, W_KR, W_Q, W_UK, W_UV, out_w, offset, T):
    """Host-side sharding/layout prep. Returns the 8 per-core input dicts."""
    f32 = np.float32
    x = np.asarray(x, f32)
    W_DKV = np.asarray(W_DKV, f32)
    kv_norm_w = np.asarray(kv_norm_w, f32)
    W_KR = np.asarray(W_KR, f32)
    W_Q = np.asarray(W_Q, f32)
    W_UK = np.asarray(W_UK, f32)
    W_UV = np.asarray(W_UV, f32)
    out_w = np.asarray(out_w, f32)
    offset = int(np.asarray(offset))

    def bf(a):
        return np.ascontiguousarray(a).astype(NPBF16)

    # rope tables, mirroring the reference's f32 arithmetic
    inv_freq = (1.0 / (THETA ** (np.arange(0, RD, 2, dtype=f32) / f32(RD)))).astype(f32)
    pos = np.arange(offset, offset + T, dtype=f32)
    ang = (pos[:, None] * inv_freq[None, :]).astype(f32)     # [T, RD/2]
    ang = np.concatenate([ang, ang], axis=-1)                # [T, RD]
    cos_t = np.cos(ang).T                                    # [RD, T]
    sin_t = np.sin(ang).T
    cosT = np.concatenate([cos_t, cos_t], 0)                 # [128, T]
    sinT = np.concatenate([sin_t, sin_t], 0)

    # signed rotate-half permutation (2 heads per 128 partitions), as lhsT
    M = np.zeros((RD, RD), f32)
    for i in range(RD // 2):
        M[i, i + RD // 2] = -1.0
        M[i + RD // 2, i] = 1.0
    perm128 = np.zeros((128, 128), f32)
    perm128[:64, :64] = M
    perm128[64:, 64:] = M
    perm_lhsT = perm128.T

    # diagonal causal masks: block r masked where (128 r + p) > f
    p_idx = np.arange(128)[:, None]
    f_idx = np.arange(512)[None, :]
    masks = np.stack(
        [(128 * r + p_idx <= f_idx).astype(f32) for r in range(4)], axis=1
    )  # [128, 4, 512]

    kvw = np.broadcast_to(kv_norm_w[None, :], (128, LAT)).astype(f32)
    ones1 = np.ones((128, 1), f32)
    onesr = np.ones((1, 128), f32)

    wuk_full = W_UK.reshape(H, HD, LAT)
    wuv_full = W_UV.reshape(H, HD, LAT)

    in_maps = []
    for b in range(2):
        xTb = bf(_part_major(x[b].T))  # [128, DC, T]
        for hg in range(4):
            hs = slice(HPC * hg * HD, HPC * (hg + 1) * HD)          # content rows
            rs = slice(D_OUT + HPC * hg * RD, D_OUT + HPC * (hg + 1) * RD)
            heads = slice(HPC * hg, HPC * (hg + 1))
            wuk_c = wuk_full[heads]                                  # [4,128,512]
            wuv_c = wuv_full[heads]
            in_maps.append(
                {
                    "xT": xTb,
                    "wdkvT": bf(_part_major(W_DKV.T)),
                    "wkrT": bf(_part_major(W_KR[HPC * hg * RD : HPC * (hg + 1) * RD].T)),
                    "wqcT": bf(_part_major(W_Q[hs].T)),
                    "wqrT": bf(_part_major(W_Q[rs].T)),
                    "wuk": bf(wuk_c.transpose(1, 0, 2)),             # [128,4,512]
                    "wuvT": bf(
                        wuv_c.transpose(0, 2, 1)                     # [4,512,128]
                        .reshape(HPC, LC, 128, HD)
                        .transpose(2, 0, 1, 3)                       # [128,4,4,128]
                    ),
                    "owT": bf(
                        out_w[:, hs].T.reshape(HPC, 128, D_OUT).transpose(1, 0, 2)
                    ),
                    "kvw": np.ascontiguousarray(kvw),
                    "cosT": bf(cosT),
                    "sinT": bf(sinT),
                    "perm": bf(perm_lhsT),
                    "masks": bf(masks),
                    "ones1": bf(ones1),
                    "onesr": np.ascontiguousarray(onesr),
                }
            )
    return in_maps


_NC_CACHE = {}


def get_nc(T=2048):
    if T not in _NC_CACHE:
        _NC_CACHE[T] = build_mla_nc(T)
    return _NC_CACHE[T]


LAST_RESULTS = None


def kernel(x, W_DKV, kv_norm_w, W_KR, W_Q, W_UK, W_UV, out_w, out_b, offset):
    global LAST_RESULTS
    import os

    x = np.asarray(x, np.float32)
    B, T, _ = x.shape
    nc = get_nc(T)
    in_maps = make_in_maps(
        x, W_DKV, kv_norm_w, W_KR, W_Q, W_UK, W_UV, out_w, offset, T
    )
    trace = os.environ.get("MLA_TRACE", "0") == "1"
    res = run_bass_kernel_spmd(
        nc, in_maps, core_ids=list(range(8)), trace=trace
    )
    LAST_RESULTS = res
    out = np.zeros((B, T, D_OUT), np.float32)
    for c, r in enumerate(res.results):
        out[c // 4] += np.asarray(r["out_p"], np.float32)
    out += np.asarray(out_b, np.float32)[None, None, :]
    return out



# revision 2
# speedup vs baseline: 1.2306x; 1.2306x over previous
"""DeepSeekV3 latent attention (MLA) Trainium2 Bass kernel.

Sharding: 8 cores = 2 batches x 4 head-groups (4 heads each).
Each core computes, for its (batch b, head-group hg):
  - c_kv = RMSNorm(x_b @ W_DKV.T) * w        (replicated across the 4 hg cores)
  - k_rope / q projections for its 4 heads (weights sliced on the head axis)
  - causal latent attention (no-max-sub softmax, exp/sum form)
  - out_partial = ctx_hg @ out_w[:, hg_cols].T   (row-parallel partial)
Host sums the 4 partials per batch and adds the bias.

Optimizations over the 738us baseline (trace-driven):
  - Softmax denominator no longer uses 160 ones-row matmuls: exp tiles are
    accumulated on DVE (bf16 adds) and ONE all-ones [128,128] matmul per
    context sums across partitions AND broadcasts in a single shot. The
    freed PSUM bank raises the score-bank rotation to 4.
  - Rope K tiles are zero-padded to full 128 rows (per-head tile, opposite
    half zeroed) so the rope LDWEIGHTS is a normal full-row load: the old
    64-row row_grp load could not overlap in-flight matmuls and cost
    ~200ns x 160 in double LDW serialization.
  - reciprocal_approx_fast (5x faster than reciprocal) reads the broadcast
    PSUM directly: kills the 3.4us PSUM-bank hostage + DVE FIFO clog at
    every context boundary and at the kernel tail.
  - qa for context i+1 is emitted before context i's last PV quad, so its
    PSUM waits and copies are fully covered; qa/cl drain copies alternate
    scalar/vector so neither FIFO gates the PE.
  - x / cos+sin DRAM layouts are contiguous-per-supertile (4KB descriptors)
    and x loads ride the idle sync queue: first matmul and HAM warmup come
    ~8us earlier.
  - wuk/wuvT/masks live in a whole-kernel pool loaded at the top of the
    gpsimd queue: phase B no longer waits on phase A pool teardown (which
    cost a 4.7us gap plus a HAM re-throttle at the seam).
  - Output DMA is one trigger per 128-token tile (4KB rows) instead of 4:
    the serialized trigger chain was most of the 5.7us tail.
Device layout: feature-on-partition, token-on-free throughout, so scores
come out as S^T [tk, tq] and probs feed the PV matmul with no transposes.
"""

import numpy as np
import ml_dtypes

import concourse.bass as bass
import concourse.tile as tile
from concourse import bacc
from concourse import mybir
from concourse.bass import ts
from concourse.bass_utils import run_bass_kernel_spmd

BF16 = mybir.dt.bfloat16
F32 = mybir.dt.float32
NPBF16 = ml_dtypes.bfloat16

H, HD, RD, LAT = 16, 128, 64, 512
D_IN = 2048
D_OUT = H * HD
HPC = 4  # heads per core
LC = LAT // 128
EPS = 1e-6
THETA = 10000.0
SCALE = 1.0 / float(np.sqrt(np.float32(HD + RD)))
AF = mybir.ActivationFunctionType
ALU = mybir.AluOpType


def build_mla_nc(T=2048):
    nc = bacc.Bacc("TRN2", target_bir_lowering=False)
    DC = D_IN // 128      # 16 contraction chunks for the x projections
    NT = T // 128         # 128-token tiles
    NJ = T // 512         # 512-token query supertiles

    # ---------------- I/O (all layouts are host-prepared, partition-major) ---
    xT = nc.dram_tensor("xT", [128, NJ, DC // 4, 4, 512], BF16, kind="ExternalInput")
    wdkvT = nc.dram_tensor("wdkvT", [128, DC, LAT], BF16, kind="ExternalInput")
    wkrT = nc.dram_tensor("wkrT", [128, DC, HPC * RD], BF16, kind="ExternalInput")
    wqcT = nc.dram_tensor("wqcT", [128, DC, HPC * HD], BF16, kind="ExternalInput")
    wqrT = nc.dram_tensor("wqrT", [128, DC, HPC * RD], BF16, kind="ExternalInput")
    wuk = nc.dram_tensor("wuk", [128, HPC, LAT], BF16, kind="ExternalInput")
    wuvT = nc.dram_tensor("wuvT", [128, HPC, LC, HD], BF16, kind="ExternalInput")
    owT = nc.dram_tensor("owT", [128, HPC, D_OUT], BF16, kind="ExternalInput")
    kvw = nc.dram_tensor("kvw", [128, LAT], F32, kind="ExternalInput")
    csT = nc.dram_tensor("csT", [128, NJ, 2, 512], BF16, kind="ExternalInput")
    perm = nc.dram_tensor("perm", [128, 128], BF16, kind="ExternalInput")
    masks = nc.dram_tensor("masks", [128, 4, 512], BF16, kind="ExternalInput")
    out_p = nc.dram_tensor("out_p", [T, D_OUT], BF16, kind="ExternalOutput")

    with tile.TileContext(nc) as tc:
        with (
            tc.tile_pool(name="persist", bufs=1) as persist,
            tc.tile_pool(name="bw", bufs=1) as bw,
        ):
            # persistent activations
            ckv_nat = persist.tile([128, NT, LAT], BF16)   # [t%128, ttile, lat]
            ckvT = persist.tile([128, NT, LC, 128], BF16)  # [lat%128, ttile, lc, tok]
            # per-head rope K, zero-padded on the opposite 64-row half so the
            # attention-phase LDWEIGHTS is a normal full-row load
            kz = persist.tile([128, HPC, T], BF16)
            qTrot = persist.tile([128, 2, T], BF16)        # [pairrow, h//2, t]
            qcT = persist.tile([128, HPC, T], BF16)        # [hd, h, t]
            ctxT = persist.tile([128, HPC, T], BF16)       # [hd, h, t]

            # attention-phase weights, loaded at the very top of the gpsimd
            # queue so the A->B seam never waits on them
            wuk_s = bw.tile([128, HPC, LAT], BF16)
            wuvT_s = bw.tile([128, HPC, LC, HD], BF16)
            masks_s = bw.tile([128, 4, 512], BF16)
            ones128 = bw.tile([128, 128], BF16)

            # zero the unused rope halves once; rope writes fill the rest
            nc.vector.memset(kz[:, :, :], 0.0)
            nc.vector.memset(ones128, 1.0)

            # ============== Phase A: projections + RMSNorm + RoPE ===========
            with (
                tc.tile_pool(name="ps_a", bufs=8, space="PSUM") as ps_a,
                tc.tile_pool(name="aw", bufs=1) as aw,
                tc.tile_pool(name="xs", bufs=2) as xs,
                tc.tile_pool(name="cs", bufs=2) as cs,
                tc.tile_pool(name="wka", bufs=2) as wka,
            ):
                wdkvT_s = aw.tile([128, DC, LAT], BF16)
                wkrT_s = aw.tile([128, DC, HPC * RD], BF16)
                wqcT_s = aw.tile([128, DC, HPC * HD], BF16)
                wqrT_s = aw.tile([128, DC, HPC * RD], BF16)
                for d4 in range(DC // 4):
                    sl = slice(4 * d4, 4 * d4 + 4)
                    nc.gpsimd.dma_start(wdkvT_s[:, sl, :], wdkvT[:, sl, :])
                for d4 in range(DC // 4):
                    sl = slice(4 * d4, 4 * d4 + 4)
                    nc.gpsimd.dma_start(wkrT_s[:, sl, :], wkrT[:, sl, :])
                    nc.gpsimd.dma_start(wqcT_s[:, sl, :], wqcT[:, sl, :])
                    nc.gpsimd.dma_start(wqrT_s[:, sl, :], wqrT[:, sl, :])
                perm_s = aw.tile([128, 128], BF16)
                nc.gpsimd.dma_start(perm_s, perm[:, :])
                kvw_s = aw.tile([128, LAT], F32)
                nc.gpsimd.dma_start(kvw_s, kvw[:, :])
                eps_s = aw.tile([128, 1], F32)
                nc.vector.memset(eps_s, EPS)
                # attention weights ride the same queue, behind the A weights
                nc.gpsimd.dma_start(wuk_s, wuk[:, :, :])
                nc.gpsimd.dma_start(wuvT_s, wuvT[:, :, :, :])
                nc.gpsimd.dma_start(masks_s, masks[:, :, :])

                def rope_pair(raw, is_k, rc, jt, cos_s, sin_s, tag):
                    # raw: [128,512] sbuf with 2 heads' raw rope rows.
                    psr = ps_a.tile([128, 512], F32, tag="mm")
                    nc.tensor.matmul(psr, lhsT=perm_s, rhs=raw, start=True, stop=True)
                    rot = wka.tile([128, 512], BF16, tag=f"{tag}_rot")
                    nc.scalar.copy(rot, psr)
                    tmp = wka.tile([128, 512], BF16, tag=f"{tag}_cos")
                    nc.vector.tensor_mul(tmp, raw, cos_s)
                    tmp2 = wka.tile([128, 512], BF16, tag=f"{tag}_sin")
                    nc.vector.tensor_mul(tmp2, rot, sin_s)
                    if is_k:
                        # per-head zero-padded tiles: head 2rc keeps the top
                        # 64 rows, head 2rc+1 the bottom 64 (rest stays 0)
                        nc.vector.tensor_add(
                            kz[0:64, 2 * rc, ts(jt, 512)], tmp[0:64, :], tmp2[0:64, :]
                        )
                        nc.vector.tensor_add(
                            kz[64:128, 2 * rc + 1, ts(jt, 512)],
                            tmp[64:128, :], tmp2[64:128, :],
                        )
                    else:
                        nc.vector.tensor_add(qTrot[:, rc, ts(jt, 512)], tmp, tmp2)

                rope_pending = []

                def flush_rope():
                    while rope_pending:
                        rope_pair(*rope_pending.pop(0))

                for jt in range(NJ):
                    xts = xs.tile([128, DC, 512], BF16)
                    for q4 in range(4):
                        nc.sync.dma_start(
                            xts[:, 4 * q4 : 4 * q4 + 4, :],
                            xT[:, jt, q4, :, :],
                        )
                    cs_t = cs.tile([128, 2, 512], BF16, tag="cs")
                    nc.scalar.dma_start(cs_t, csT[:, jt, :, :])
                    cos_s = cs_t[:, 0, :]
                    sin_s = cs_t[:, 1, :]

                    # --- c_kv (natural layout) + RMSNorm ---
                    for tt4 in range(4):
                        tt = jt * 4 + tt4
                        ps = ps_a.tile([128, 512], F32, tag="mm")
                        for dc in range(DC):
                            nc.tensor.matmul(
                                ps,
                                lhsT=xts[:, dc, ts(tt4, 128)],
                                rhs=wdkvT_s[:, dc, :],
                                start=(dc == 0),
                                stop=(dc == DC - 1),
                            )
                        sq = wka.tile([128, LAT], BF16, tag="sq")
                        ssum = wka.tile([128, 1], F32, tag="ssum")
                        nc.scalar.activation(sq, ps, AF.Square, accum_out=ssum)
                        rstd = wka.tile([128, 1], F32, tag="rstd")
                        nc.scalar.activation(
                            rstd, ssum, AF.Sqrt, bias=eps_s, scale=1.0 / LAT
                        )
                        nc.vector.reciprocal(rstd, rstd)
                        nc.vector.scalar_tensor_tensor(
                            ckv_nat[:, tt, :], ps, rstd, kvw_s,
                            op0=ALU.mult, op1=ALU.mult,
                        )
                        # transposed copy for the QK side (single xbar
                        # transpose per token tile; contiguous destination)
                        nc.sync.dma_start_transpose(
                            ckvT[:, tt, :, :], ckv_nat[:, tt, :]
                        )

                    # --- rope + q projections, rc0 (heads 0/1) first so the
                    #     woven attention contexts unblock as early as possible
                    def proj_group(w_s, col):
                        ps = ps_a.tile([128, 512], F32, tag="mm")
                        for dc in range(DC):
                            nc.tensor.matmul(
                                ps,
                                lhsT=w_s[:, dc, ts(col, 128)],
                                rhs=xts[:, dc, :],
                                start=(dc == 0),
                                stop=(dc == DC - 1),
                            )
                        return ps

                    for rc in range(2):
                        ps = proj_group(wkrT_s, rc)
                        raw = wka.tile([128, 512], BF16, tag="k_raw")
                        nc.scalar.copy(raw, ps)
                        rope_pending.append((raw, True, rc, jt, cos_s, sin_s, "k"))
                        ps = proj_group(wqrT_s, rc)
                        raw = wka.tile([128, 512], BF16, tag="q_raw")
                        nc.scalar.copy(raw, ps)
                        rope_pending.append((raw, False, rc, jt, cos_s, sin_s, "q"))
                        for fc in (range(2) if rc == 0 else range(2, HPC)):
                            ps = proj_group(wqcT_s, fc)
                            nc.scalar.copy(qcT[:, fc, ts(jt, 512)], ps)
                            flush_rope()
                flush_rope()

            # ============== Phase B: attention, j outer / head inner ========
            with (
                tc.tile_pool(name="ps_s", bufs=4, space="PSUM") as ps_s,
                tc.tile_pool(name="ps_pv", bufs=1, space="PSUM") as ps_pv,
                tc.tile_pool(name="bw2", bufs=1) as bw2,
                tc.tile_pool(name="qa", bufs=3) as qa_pool,
                tc.tile_pool(name="exps", bufs=12) as exps,
                tc.tile_pool(name="wkb", bufs=3) as wkb,
                tc.tile_pool(name="dnp", bufs=2) as dnp,
                tc.tile_pool(name="outs", bufs=2) as outs,
            ):
                owT_s = bw2.tile([128, HPC, D_OUT], BF16)
                for hc4 in range(HPC):
                    nc.gpsimd.dma_start(owT_s[:, hc4, :], owT[:, hc4, :])

                def emit_qa(j, h):
                    qa_t = qa_pool.tile([128, LC, 512], BF16, tag="qa")
                    for lc in range(LC):
                        ps = ps_s.tile([128, 512], F32, tag="sc")
                        nc.tensor.matmul(
                            ps,
                            lhsT=wuk_s[:, h, ts(lc, 128)],
                            rhs=qcT[:, h, ts(j, 512)],
                            start=True,
                            stop=True,
                        )
                        # alternate engines so neither FIFO gates the copies
                        if lc % 2 == 0:
                            nc.vector.tensor_copy(qa_t[:, lc, :], ps)
                        else:
                            nc.scalar.copy(qa_t[:, lc, :], ps)
                    return qa_t

                def emit_qk_quad(j, h, quad, qa_t, dn_acc):
                    nquad = j + 1
                    exs = []
                    for tq in range(4):
                        tk = 4 * quad + tq
                        ps = ps_s.tile([128, 512], F32, tag="sc")
                        for lc in range(LC):
                            nc.tensor.matmul(
                                ps,
                                lhsT=ckvT[:, tk, lc, :],
                                rhs=qa_t[:, lc, :],
                                start=(lc == 0),
                                stop=False,
                            )
                        nc.tensor.matmul(
                            ps,
                            lhsT=kz[:, h, ts(tk, 128)],
                            rhs=qTrot[:, h // 2, ts(j, 512)],
                            start=False,
                            stop=True,
                        )
                        ex = exps.tile([128, 512], BF16, tag="exp")
                        nc.scalar.activation(ex, ps, AF.Exp, scale=SCALE)
                        if quad == nquad - 1:
                            nc.vector.tensor_mul(ex, ex, masks_s[:, tq, :])
                        # softmax denominator: accumulate exp tiles on DVE
                        # (replaces a ones-row matmul per tile)
                        if quad == 0 and tq == 0:
                            nc.vector.tensor_copy(dn_acc, ex)
                        else:
                            nc.vector.tensor_add(dn_acc, dn_acc, ex)
                        exs.append(ex)
                    return exs

                def emit_pv_quad(j, quad, exs, ps_ctx):
                    ntk = 4 * (j + 1)
                    for tq in range(4):
                        tk = 4 * quad + tq
                        for lc in range(LC):
                            nc.tensor.matmul(
                                ps_ctx[:, lc, :],
                                lhsT=ckv_nat[:, tk, ts(lc, 128)],
                                rhs=exs[tq],
                                start=(tk == 0),
                                stop=(tk == ntk - 1),
                            )

                def emit_dn_bcast(dn_acc):
                    # one matmul sums the 128 partition-partials AND
                    # broadcasts the result across all partitions
                    ps_bc = ps_s.tile([128, 512], F32, tag="sc")
                    nc.tensor.matmul(
                        ps_bc, lhsT=ones128, rhs=dn_acc, start=True, stop=True
                    )
                    return ps_bc

                def emit_drain_casts(ps_ctx):
                    cl = wkb.tile([128, LC, 512], BF16, tag="ctxlat")
                    for lc in range(LC):
                        if lc % 2 == 0:
                            nc.scalar.copy(cl[:, lc, :], ps_ctx[:, lc, :])
                        else:
                            nc.vector.tensor_copy(cl[:, lc, :], ps_ctx[:, lc, :])
                    return cl

                def emit_drain_rest(j, h, cl, ps_bc):
                    db = wkb.tile([128, 512], F32, tag="db")
                    nc.vector.reciprocal_approx_fast(db, ps_bc[:, :])
                    ps_uv = ps_s.tile([128, 512], F32, tag="sc")
                    for lc in range(LC):
                        nc.tensor.matmul(
                            ps_uv,
                            lhsT=wuvT_s[:, h, lc, :],
                            rhs=cl[:, lc, :],
                            start=(lc == 0),
                            stop=(lc == LC - 1),
                        )
                    nc.vector.tensor_mul(ctxT[:, h, ts(j, 512)], ps_uv, db)

                def emit_out_proj(j):
                    for tt4 in range(4):
                        tt = 4 * j + tt4
                        ot = outs.tile([128, D_OUT], BF16, tag="ot")
                        for oc in range(D_OUT // 512):
                            ps = ps_s.tile([128, 512], F32, tag="sc")
                            for hc in range(HPC):
                                nc.tensor.matmul(
                                    ps,
                                    lhsT=ctxT[:, hc, ts(tt, 128)],
                                    rhs=owT_s[:, hc, ts(oc, 512)],
                                    start=(hc == 0),
                                    stop=(hc == HPC - 1),
                                )
                            if oc % 2 == 0:
                                nc.scalar.copy(ot[:, ts(oc, 512)], ps)
                            else:
                                nc.vector.tensor_copy(ot[:, ts(oc, 512)], ps)
                        # one DMA per 128-token tile (4KB rows): the old
                        # per-chunk triggers serialized on the sync engine
                        nc.sync.dma_start(out_p[ts(tt, 128), :], ot[:, :])

                # software-pipelined emission: drains of context i-1 ride
                # behind context i's first QK quad; qa for context i+1 is
                # emitted before context i's last PV quad.
                # j1 before j0: j0's shallow contexts then sit where j2's
                # deep quads cover their drains
                j_order = [1, 0] + list(range(2, NJ))
                contexts = [(j, h) for j in j_order for h in range(HPC)]
                pending = None       # (j, h, ps_ctx, dn_acc)
                pending_out = None
                next_qa = emit_qa(*contexts[0])
                for idx, (j, h) in enumerate(contexts):
                    qa_t = next_qa
                    next_qa = None
                    cl = None
                    if pending is not None:
                        cl = emit_drain_casts(pending[2])
                    dn_acc = dnp.tile([128, 512], BF16, tag="dn")
                    exs = emit_qk_quad(j, h, 0, qa_t, dn_acc)
                    if pending is not None:
                        ps_bc = emit_dn_bcast(pending[3])
                    if pending_out is not None:
                        emit_out_proj(pending_out)
                        pending_out = None
                    if pending is not None:
                        emit_drain_rest(pending[0], pending[1], cl, ps_bc)
                        if pending[1] == HPC - 1:
                            pending_out = pending[0]
                        pending = None
                    ps_ctx = ps_pv.tile([128, LC, 512], F32, tag="pv")
                    for quad in range(j + 1):
                        if quad > 0:
                            exs = emit_qk_quad(j, h, quad, qa_t, dn_acc)
                        if quad == j and idx + 1 < len(contexts):
                            next_qa = emit_qa(*contexts[idx + 1])
                        emit_pv_quad(j, quad, exs, ps_ctx)
                    pending = (j, h, ps_ctx, dn_acc)
                cl = emit_drain_casts(pending[2])
                ps_bc = emit_dn_bcast(pending[3])
                emit_drain_rest(pending[0], pending[1], cl, ps_bc)
                emit_out_proj(NJ - 1)

    nc.finalize()
    return nc


def _part_major(a2d):
    """[R, C] -> [128, R//128, C] with partition = R % 128."""
    r, c = a2d.shape
    return np.ascontiguousarray(
        a2d.reshape(r // 128, 128, c).transpose(1, 0, 2)
    )


def make_in_maps(x, W_DKV, kv_norm_w, W_KR, W_Q, W_UK, W_UV, out_w, offset, T):
    """Host-side sharding/layout prep. Returns the 8 per-core input dicts."""
    f32 = np.float32
    x = np.asarray(x, f32)
    W_DKV = np.asarray(W_DKV, f32)
    kv_norm_w = np.asarray(kv_norm_w, f32)
    W_KR = np.asarray(W_KR, f32)
    W_Q = np.asarray(W_Q, f32)
    W_UK = np.asarray(W_UK, f32)
    W_UV = np.asarray(W_UV, f32)
    out_w = np.asarray(out_w, f32)
    offset = int(np.asarray(offset))
    DC = D_IN // 128
    NJ = T // 512

    def bf(a):
        return np.ascontiguousarray(a).astype(NPBF16)

    # rope tables, mirroring the reference's f32 arithmetic
    inv_freq = (1.0 / (THETA ** (np.arange(0, RD, 2, dtype=f32) / f32(RD)))).astype(f32)
    pos = np.arange(offset, offset + T, dtype=f32)
    ang = (pos[:, None] * inv_freq[None, :]).astype(f32)     # [T, RD/2]
    ang = np.concatenate([ang, ang], axis=-1)                # [T, RD]
    cos_t = np.cos(ang).T                                    # [RD, T]
    sin_t = np.sin(ang).T
    cosT = np.concatenate([cos_t, cos_t], 0)                 # [128, T]
    sinT = np.concatenate([sin_t, sin_t], 0)
    # [128, NJ, 2, 512]: per-supertile contiguous cos+sin
    csT = np.stack(
        [cosT.reshape(128, NJ, 512), sinT.reshape(128, NJ, 512)], axis=2
    )

    # signed rotate-half permutation (2 heads per 128 partitions), as lhsT
    M = np.zeros((RD, RD), f32)
    for i in range(RD // 2):
        M[i, i + RD // 2] = -1.0
        M[i + RD // 2, i] = 1.0
    perm128 = np.zeros((128, 128), f32)
    perm128[:64, :64] = M
    perm128[64:, 64:] = M
    perm_lhsT = perm128.T

    # diagonal causal masks: block r masked where (128 r + p) > f
    p_idx = np.arange(128)[:, None]
    f_idx = np.arange(512)[None, :]
    masks = np.stack(
        [(128 * r + p_idx <= f_idx).astype(f32) for r in range(4)], axis=1
    )  # [128, 4, 512]

    kvw = np.broadcast_to(kv_norm_w[None, :], (128, LAT)).astype(f32)

    wuk_full = W_UK.reshape(H, HD, LAT)
    wuv_full = W_UV.reshape(H, HD, LAT)

    in_maps = []
    for b in range(2):
        xTb = _part_major(x[b].T)                            # [128, DC, T]
        # [128, NJ, DC//4, 4, 512]: per-(supertile, dc-quad) contiguous
        xTb = bf(
            xTb.reshape(128, DC // 4, 4, NJ, 512).transpose(0, 3, 1, 2, 4)
        )
        for hg in range(4):
            hs = slice(HPC * hg * HD, HPC * (hg + 1) * HD)          # content rows
            rs = slice(D_OUT + HPC * hg * RD, D_OUT + HPC * (hg + 1) * RD)
            heads = slice(HPC * hg, HPC * (hg + 1))
            wuk_c = wuk_full[heads]                                  # [4,128,512]
            wuv_c = wuv_full[heads]
            in_maps.append(
                {
                    "xT": xTb,
                    "wdkvT": bf(_part_major(W_DKV.T)),
                    "wkrT": bf(_part_major(W_KR[HPC * hg * RD : HPC * (hg + 1) * RD].T)),
                    "wqcT": bf(_part_major(W_Q[hs].T)),
                    "wqrT": bf(_part_major(W_Q[rs].T)),
                    "wuk": bf(wuk_c.transpose(1, 0, 2)),             # [128,4,512]
                    "wuvT": bf(
                        wuv_c.transpose(0, 2, 1)                     # [4,512,128]
                        .reshape(HPC, LC, 128, HD)
                        .transpose(2, 0, 1, 3)                       # [128,4,4,128]
                    ),
                    "owT": bf(
                        out_w[:, hs].T.reshape(HPC, 128, D_OUT).transpose(1, 0, 2)
                    ),
                    "kvw": np.ascontiguousarray(kvw),
                    "csT": bf(csT),
                    "perm": bf(perm_lhsT),
                    "masks": bf(masks),
                }
            )
    return in_maps


_NC_CACHE = {}


def get_nc(T=2048):
    if T not in _NC_CACHE:
        _NC_CACHE[T] = build_mla_nc(T)
    return _NC_CACHE[T]


LAST_RESULTS = None


def kernel(x, W_DKV, kv_norm_w, W_KR, W_Q, W_UK, W_UV, out_w, out_b, offset):
    global LAST_RESULTS
    import os

    x = np.asarray(x, np.float32)
    B, T, _ = x.shape
    nc = get_nc(T)
    in_maps = make_in_maps(
        x, W_DKV, kv_norm_w, W_KR, W_Q, W_UK, W_UV, out_w, offset, T
    )
    trace = os.environ.get("MLA_TRACE", "0") == "1"
    res = run_bass_kernel_spmd(
        nc, in_maps, core_ids=list(range(8)), trace=trace
    )
    LAST_RESULTS = res
    out = np.zeros((B, T, D_OUT), np.float32)
    for c, r in enumerate(res.results):
        out[c // 4] += np.asarray(r["out_p"], np.float32)
    out += np.asarray(out_b, np.float32)[None, None, :]
    return out
